# revision 8
# baseline (speedup 1.0000x reference)
"""Trainium2 Bass kernel for nn_GAT_GraphSAGE (N=12000, E=192000, F=35, B=64).

Sharding: the attention "row" dimension (K_new index i, which is also the
softmax row) is sharded 1500 rows/core across 8 cores.  Q and V are computed
replicated (cheap [N,35] projections), so the only collective is one
AllGather of the post-attention node features h.  SAGEConv is sharded by dst
node (same 1500-row shard): h[src] rows are fetched with one batched
dma_gather and scatter-added via one-hot matmuls in PSUM.  Global max-pool +
MLP head run per-core on that core's 8 graphs (graph boundaries align with
the 1500-row shard exactly).

The whole K-branch (Wk/conv-center-taps/Wl/1-sqrt(F)) folds on the host into
a single affine [35->35] map; biases ride an appended ones-row of x^T.
"""
import math
import numpy as np

N, E, F, B = 12000, 192000, 35, 64
NCORE = 8
ROWS = N // NCORE            # 1500
ICH = 512
NI = 3
IPAD = ICH * NI              # 1536
JT = 94                      # j chunks of 128
JPAD = JT * 128              # 12032
XW = 12064                   # padded x^T width (covers 7*1500 + 1536)
DBLK = 12                    # dst blocks (128 each) per core
GB = B // NCORE              # 8 graphs per core
HPAD = 64                    # h row padded to 64 f32 (256B) for dma_gather
GRAPH_BOUNDS = [int(math.ceil(g * (N / B))) for g in range(GB + 1)]
F1 = F + 1


# --------------------------------------------------------------------------
# host-side preprocessing
# --------------------------------------------------------------------------

def _prep_weights(p):
    f64 = np.float64
    f32 = np.float32
    Wq, bq = p['Wq'].astype(f64), p['bq'].astype(f64)
    Wk, bk = p['Wk'].astype(f64), p['bk'].astype(f64)
    Wv, bv = p['Wv'].astype(f64), p['bv'].astype(f64)
    W3c, b3 = p['W3'][:, :, 1].astype(f64), p['b3'].astype(f64)
    W5c, b5 = p['W5'][:, :, 2].astype(f64), p['b5'].astype(f64)
    Wl, bl = p['Wl'].astype(f64), p['bl'].astype(f64)
    Wl1, Wl2, Wl3 = Wl[:, :F], Wl[:, F:2 * F], Wl[:, 2 * F:]

    Weff = W3c.T @ Wl1.T + W5c.T @ Wl2.T + Wl3.T
    beff = b3 @ Wl1.T + b5 @ Wl2.T + bl
    Wkn = Wk.T @ Weff
    bkn = bk @ Weff + beff
    s = 1.0 / np.sqrt(F)
    Wkn, bkn = Wkn * s, bkn * s

    out = {}
    out['Wq_h'] = np.vstack([Wq.T, bq[None, :]]).astype(f32)
    out['Wkn_h'] = np.vstack([Wkn, bkn[None, :]]).astype(f32)
    out['Wv_h'] = np.vstack([Wv.T, bv[None, :]]).astype(f32)
    wva = np.zeros((F1, F1))
    wva[:, :F] = out['Wv_h']
    wva[F, F] = 1.0
    out['Wv_aug'] = wva.astype(f32)
    out['WllT'] = np.ascontiguousarray(p['Wll'].T).astype(f32)
    out['WlrT'] = np.ascontiguousarray(p['Wlr'].T).astype(f32)
    out['bll'] = p['bll'].astype(f32).reshape(F, 1)
    out['Wg1T'] = np.ascontiguousarray(p['Wg1'].T).astype(f32)      # [35,1500]
    bg1 = np.zeros((128, 12), f32)
    bg1.T.reshape(-1)[:1500] = p['bg1'].astype(f32)
    out['bg1'] = bg1
    w2 = np.zeros((12 * 128, 128), f32)
    w2[:1500, :] = p['Wg2'].T.astype(f32)
    out['Wg2Tr'] = np.ascontiguousarray(
        w2.reshape(12, 128, 128).transpose(1, 0, 2).reshape(128, 12 * 128))
    out['bg2'] = p['bg2'].astype(f32).reshape(128, 1)
    out['WoT'] = p['Wo'].astype(f32).reshape(1, 128).T.copy()        # [128,1]
    out['bo'] = float(np.asarray(p['bo']).reshape(-1)[0])
    return out


def _prep_x(x):
    xhT = np.zeros((F1, XW), np.float32)
    xhT[:F, :N] = np.asarray(x, np.float32).T
    xhT[F, :] = 1.0
    xl = [np.ascontiguousarray(xhT[:, c * ROWS: c * ROWS + IPAD])
          for c in range(NCORE)]
    return xhT, xl


def _prep_edges(edge_index):
    src = np.asarray(edge_index[0], np.int64)
    dst = np.asarray(edge_index[1], np.int64)
    deg = np.bincount(dst, minlength=N).astype(np.float64)
    recip = (1.0 / np.maximum(deg, 1.0)).astype(np.float32)

    core_of = dst // ROWS
    blk_of = (dst - core_of * ROWS) // 128
    counts = np.zeros((NCORE, DBLK), np.int64)
    np.add.at(counts, (core_of, blk_of), 1)
    S = int(np.ceil(counts.max() / 128))

    order = np.lexsort((dst,))
    src_s, dst_s = src[order], dst[order]
    core_s, blk_s = core_of[order], blk_of[order]

    gidx, dstrel = [], []
    for c in range(NCORE):
        idx_c = np.zeros(DBLK * S * 128, np.int16)
        rel_c = np.full(DBLK * S * 128, -1.0, np.float32)
        m_c = core_s == c
        for b in range(DBLK):
            m = m_c & (blk_s == b)
            n = int(m.sum())
            lo = b * S * 128
            idx_c[lo:lo + n] = src_s[m].astype(np.int16)
            rel_c[lo:lo + n] = (dst_s[m] - c * ROWS - b * 128).astype(np.float32)
        # HW convention: the [16, n] packed index block must be replicated
        # across all eight 16-partition groups (sim reads only rows 0:16).
        gidx.append(np.ascontiguousarray(
            np.tile(idx_c.reshape(-1, 16).T, (8, 1))))
        dstrel.append(np.ascontiguousarray(rel_c.reshape(-1, 128).T))

    recipT = []
    for c in range(NCORE):
        r = np.ones(IPAD, np.float32)
        r[:ROWS] = recip[c * ROWS:(c + 1) * ROWS]
        recipT.append(np.ascontiguousarray(np.broadcast_to(r, (F, IPAD))))
    return gidx, dstrel, recipT, S


# --------------------------------------------------------------------------
# device program
# --------------------------------------------------------------------------

def _build_program(S, bo_const):
    import concourse.bass as bass
    import concourse.tile as tile
    from concourse import bacc, mybir
    from concourse.bass_utils import axon_active

    f32 = mybir.dt.float32
    nc = bacc.Bacc("TRN2", target_bir_lowering=False, debug=False,
                   num_devices=NCORE)

    din = {}

    def dram_in(name, shape, dt=f32):
        din[name] = nc.dram_tensor(name, list(shape), dt, kind="ExternalInput")
        return din[name]

    xhT_d = dram_in("xhT", (F1, XW))
    xlT_d = dram_in("xlT", (F1, IPAD))
    Wq_d = dram_in("Wq_h", (F1, F))
    Wkn_d = dram_in("Wkn_h", (F1, F))
    Wv_d = dram_in("Wv_h", (F1, F))
    Wva_d = dram_in("Wv_aug", (F1, F1))
    WllT_d = dram_in("WllT", (F, F))
    WlrT_d = dram_in("WlrT", (F, F))
    bll_d = dram_in("bll", (F, 1))
    Wg1T_d = dram_in("Wg1T", (F, 1500))
    bg1_d = dram_in("bg1", (128, 12))
    Wg2_d = dram_in("Wg2Tr", (128, 12 * 128))
    bg2_d = dram_in("bg2", (128, 1))
    WoT_d = dram_in("WoT", (128, 1))
    recT_d = dram_in("recipT", (F, IPAD))
    iota_d = dram_in("iota", (128, 128))
    ident_d = dram_in("ident", (128, 128))
    gidx_d = dram_in("gidx", (128, DBLK * S * 8), mybir.dt.int16)
    drel_d = dram_in("dstrel", (128, DBLK * S))

    out_d = nc.dram_tensor("out8", [1, GB], f32, kind="ExternalOutput")

    h_loc = nc.dram_tensor("h_loc", [ROWS, HPAD], f32)
    h_full = nc.dram_tensor("h_full", [N, HPAD], f32, addr_space="Shared")

    NIDX = DBLK * S * 128

    with tile.TileContext(nc) as tc:
        with tc.tile_pool(name="const", bufs=1) as constp, \
             tc.tile_pool(name="main", bufs=1) as main:
            # ---- constants / small weights ----
            Wq_t = constp.tile([F1, F], f32)
            nc.sync.dma_start(out=Wq_t[:], in_=Wq_d[:, :])
            Wkn_t = constp.tile([F1, F], f32)
            nc.sync.dma_start(out=Wkn_t[:], in_=Wkn_d[:, :])
            Wv_t = constp.tile([F1, F], f32)
            nc.sync.dma_start(out=Wv_t[:], in_=Wv_d[:, :])
            Wva_t = constp.tile([F1, F1], f32)
            nc.sync.dma_start(out=Wva_t[:], in_=Wva_d[:, :])
            ident_t = constp.tile([128, 128], f32)
            nc.sync.dma_start(out=ident_t[:], in_=ident_d[:, :])

            # ---- big persistent sbuf tensors ----
            QT = main.tile([F, JPAD], f32)
            KnT = main.tile([F, IPAD], f32)
            Vp = main.tile([128, JT, F1], f32)          # V' natural, full
            Vl = main.tile([128, DBLK, F], f32)         # V natural, local rows
            hnat = main.tile([128, DBLK, HPAD], f32)    # h natural, local rows

            with tc.tile_pool(name="prep", bufs=2, space="PSUM") as pp, \
                 tc.tile_pool(name="prepin", bufs=1) as pin, \
                 tc.tile_pool(name="prepsb", bufs=3) as psb:
                xhT_t = pin.tile([F1, XW], f32)
                nc.sync.dma_start(out=xhT_t[:], in_=xhT_d[:, :])
                xlT_t = pin.tile([F1, IPAD], f32)
                nc.sync.dma_start(out=xlT_t[:], in_=xlT_d[:, :])

                # QT full: 24 matmuls over 512-chunks (covers JPAD=12032)
                nq = JPAD // ICH  # 23.5 -> handle tail
                for ci in range((JPAD + ICH - 1) // ICH):
                    w = min(ICH, JPAD - ci * ICH)
                    ps = pp.tile([F, ICH], f32, space="PSUM", tag="ppq")
                    nc.tensor.matmul(out=ps[:, :w], lhsT=Wq_t[:],
                                     rhs=xhT_t[:, ci * ICH: ci * ICH + w],
                                     start=True, stop=True)
                    nc.vector.tensor_copy(out=QT[:, ci * ICH: ci * ICH + w],
                                          in_=ps[:, :w])
                # K_newT local
                for ci in range(NI):
                    ps = pp.tile([F, ICH], f32, space="PSUM", tag="ppq")
                    nc.tensor.matmul(out=ps[:], lhsT=Wkn_t[:],
                                     rhs=xlT_t[:, ci * ICH:(ci + 1) * ICH],
                                     start=True, stop=True)
                    nc.vector.tensor_copy(out=KnT[:, ci * ICH:(ci + 1) * ICH],
                                          in_=ps[:])
                # V natural local rows (12 x [128,35])
                for t in range(DBLK):
                    ps = pp.tile([128, F], f32, space="PSUM", tag="ppv")
                    nc.tensor.matmul(out=ps[:], lhsT=xlT_t[:, t * 128:(t + 1) * 128],
                                     rhs=Wv_t[:], start=True, stop=True)
                    nc.vector.tensor_copy(out=Vl[:, t, :], in_=ps[:])
                # V' natural full (94 x [128,36]); zero the 32 pad rows of
                # the last chunk (j in [12000,12032))
                for j in range(JT):
                    ps = pp.tile([128, F1], f32, space="PSUM", tag="ppv")
                    nc.tensor.matmul(out=ps[:], lhsT=xhT_t[:, j * 128:(j + 1) * 128],
                                     rhs=Wva_t[:], start=True, stop=True)
                    if j == JT - 1:
                        nc.vector.tensor_copy(out=Vp[:96, j, :], in_=ps[:96, :])
                        nc.vector.memset(Vp[96:128, j, :], 0.0)
                    else:
                        nc.vector.tensor_copy(out=Vp[:, j, :], in_=ps[:])

            # ---------------- attention ----------------
            with tc.tile_pool(name="mm1p", bufs=3, space="PSUM") as mm1p, \
                 tc.tile_pool(name="Up", bufs=2, space="PSUM") as Upp, \
                 tc.tile_pool(name="tp", bufs=2, space="PSUM") as tpp, \
                 tc.tile_pool(name="esb", bufs=4) as esb, \
                 tc.tile_pool(name="usb", bufs=2) as usb, \
                 tc.tile_pool(name="hsmall", bufs=4) as hsmall:
                exp_f = mybir.ActivationFunctionType.Exp
                for ci in range(NI):
                    Ups = Upp.tile([F1, ICH], f32, space="PSUM", tag="U")
                    prev = None  # (exp_tile, j)
                    for j in range(JT):
                        ps = mm1p.tile([128, ICH], f32, space="PSUM", tag="s")
                        nc.tensor.matmul(out=ps[:], lhsT=QT[:, j * 128:(j + 1) * 128],
                                         rhs=KnT[:, ci * ICH:(ci + 1) * ICH],
                                         start=True, stop=True)
                        et = esb.tile([128, ICH], f32, tag="e")
                        nc.scalar.activation(out=et[:], in_=ps[:], func=exp_f)
                        if prev is not None:
                            pe, pj = prev
                            nc.tensor.matmul(out=Ups[:], lhsT=Vp[:, pj, :],
                                             rhs=pe[:], start=(pj == 0),
                                             stop=False, skip_group_check=True)
                        prev = (et, j)
                    pe, pj = prev
                    nc.tensor.matmul(out=Ups[:], lhsT=Vp[:, pj, :], rhs=pe[:],
                                     start=False, stop=True, skip_group_check=True)
                    # normalize + residual + relu -> h natural tiles
                    Usb = usb.tile([F1, ICH], f32, tag="usb")
                    nc.vector.tensor_copy(out=Usb[:], in_=Ups[:])
                    for t in range(4):
                        blk = ci * 4 + t
                        up = tpp.tile([128, F1], f32, space="PSUM", tag="unat")
                        nc.tensor.transpose(out=up[:], in_=Usb[:, t * 128:(t + 1) * 128],
                                            identity=ident_t[:F1, :F1])
                        rec = hsmall.tile([128, 1], f32, tag="rec")
                        nc.vector.reciprocal(out=rec[:], in_=up[:, F:F1])
                        hh = hsmall.tile([128, F], f32, tag="hh")
                        nc.vector.scalar_tensor_tensor(
                            out=hh[:], in0=up[:, :F], scalar=rec[:],
                            in1=Vl[:, blk, :], op0=mybir.AluOpType.mult,
                            op1=mybir.AluOpType.add)
                        nc.vector.tensor_scalar_max(out=hnat[:, blk, :F], in0=hh[:],
                                                    scalar1=0.0)
                        nc.vector.memset(hnat[:, blk, F:HPAD], 0.0)
                        # store valid rows to DRAM for the AllGather
                        lo = blk * 128
                        nrows = min(128, max(0, ROWS - lo))
                        if nrows > 0:
                            nc.sync.dma_start(
                                out=h_loc[lo:lo + nrows, :],
                                in_=hnat[:nrows, blk, :])

            # hT local (for SAGE lin_r): transpose the 12 h tiles
            hT = main.tile([F, IPAD], f32)
            with tc.tile_pool(name="htp", bufs=2, space="PSUM") as htp:
                for t in range(DBLK):
                    ps = htp.tile([F, 128], f32, space="PSUM", tag="ht")
                    nc.tensor.transpose(out=ps[:], in_=hnat[:, t, :F],
                                        identity=ident_t[:])
                    nc.vector.tensor_copy(out=hT[:, t * 128:(t + 1) * 128], in_=ps[:])

            # ---------------- AllGather h ----------------
            nc.gpsimd.collective_compute(
                "AllGather", mybir.AluOpType.bypass,
                replica_groups=[list(range(NCORE))],
                ins=[h_loc[:, :]], outs=[h_full[:, :]])

            # ---------------- SAGE scatter ----------------
            aggdT = main.tile([F, IPAD], f32)
            h2T = main.tile([F, IPAD], f32)
            with tc.tile_pool(name="gat", bufs=1) as gat, \
                 tc.tile_pool(name="sca", bufs=4) as sca, \
                 tc.tile_pool(name="scp", bufs=2, space="PSUM") as scp, \
                 tc.tile_pool(name="sin", bufs=1) as sin:
                iota_t = sin.tile([128, 128], f32)
                nc.sync.dma_start(out=iota_t[:], in_=iota_d[:, :])
                drel_t = sin.tile([128, DBLK * S], f32)
                nc.sync.dma_start(out=drel_t[:], in_=drel_d[:, :])
                idx_t = sin.tile([128, DBLK * S * 8], mybir.dt.int16)
                nc.sync.dma_start(out=idx_t[:], in_=gidx_d[:, :])
                recT_t = sin.tile([F, IPAD], f32)
                nc.sync.dma_start(out=recT_t[:], in_=recT_d[:, :])

                G = gat.tile([128, DBLK * S, HPAD], f32)
                # split the gather to pipeline SWDGE/SDMA with the matmuls
                GSPLIT = 4
                assert (DBLK * S) % GSPLIT == 0
                cpg = DBLK * S // GSPLIT          # chunks per gather
                for g in range(GSPLIT):
                    nc.gpsimd.dma_gather(
                        out_ap=G[:, g * cpg:(g + 1) * cpg, :],
                        in_ap=h_full[:, :],
                        idxs_ap=idx_t[:, g * cpg * 8:(g + 1) * cpg * 8],
                        num_idxs=cpg * 128,
                        num_idxs_reg=cpg * 128,
                        elem_size=HPAD,
                        single_packet=False)

                for b in range(DBLK):
                    acc = scp.tile([F, 128], f32, space="PSUM", tag="agg")
                    for s in range(S):
                        ch = b * S + s
                        P = sca.tile([128, 128], f32, tag="P")
                        nc.vector.tensor_scalar(
                            out=P[:], in0=iota_t[:],
                            scalar1=drel_t[:, ch:ch + 1], scalar2=None,
                            op0=mybir.AluOpType.is_equal)
                        nc.tensor.matmul(out=acc[:], lhsT=G[:, ch, :F], rhs=P[:],
                                         start=(s == 0), stop=(s == S - 1),
                                         skip_group_check=True)
                    nc.vector.tensor_mul(out=aggdT[:, b * 128:(b + 1) * 128],
                                         in0=acc[:],
                                         in1=recT_t[:, b * 128:(b + 1) * 128])

            # ---------------- SAGE linear + pool + MLP ----------------
            with tc.tile_pool(name="mlpw", bufs=1) as mlpw, \
                 tc.tile_pool(name="mlps", bufs=2) as mlps, \
                 tc.tile_pool(name="mlpp", bufs=2, space="PSUM") as mlpp:
                WllT_t = mlpw.tile([F, F], f32)
                nc.sync.dma_start(out=WllT_t[:], in_=WllT_d[:, :])
                WlrT_t = mlpw.tile([F, F], f32)
                nc.sync.dma_start(out=WlrT_t[:], in_=WlrT_d[:, :])
                bll_t = mlpw.tile([F, 1], f32)
                nc.sync.dma_start(out=bll_t[:], in_=bll_d[:, :])
                Wg1T_t = mlpw.tile([F, 1500], f32)
                nc.sync.dma_start(out=Wg1T_t[:], in_=Wg1T_d[:, :])
                bg1_t = mlpw.tile([128, 12], f32)
                nc.sync.dma_start(out=bg1_t[:], in_=bg1_d[:, :])
                Wg2_t = mlpw.tile([128, 12 * 128], f32)
                nc.sync.dma_start(out=Wg2_t[:], in_=Wg2_d[:, :])
                bg2_t = mlpw.tile([128, 1], f32)
                nc.sync.dma_start(out=bg2_t[:], in_=bg2_d[:, :])
                WoT_t = mlpw.tile([128, 1], f32)
                nc.sync.dma_start(out=WoT_t[:], in_=WoT_d[:, :])

                relu_f = mybir.ActivationFunctionType.Relu
                for ci in range(NI):
                    ps = mlpp.tile([F, ICH], f32, space="PSUM", tag="h2")
                    nc.tensor.matmul(out=ps[:], lhsT=WllT_t[:],
                                     rhs=aggdT[:, ci * ICH:(ci + 1) * ICH],
                                     start=True, stop=False, skip_group_check=True)
                    nc.tensor.matmul(out=ps[:], lhsT=WlrT_t[:],
                                     rhs=hT[:, ci * ICH:(ci + 1) * ICH],
                                     start=False, stop=True, skip_group_check=True)
                    nc.scalar.activation(out=h2T[:, ci * ICH:(ci + 1) * ICH],
                                         in_=ps[:], func=relu_f, bias=bll_t[:])

                gT = mlps.tile([F, GB], f32)
                for g in range(GB):
                    lo, hi = GRAPH_BOUNDS[g], GRAPH_BOUNDS[g + 1]
                    nc.vector.tensor_reduce(out=gT[:, g:g + 1], in_=h2T[:, lo:hi],
                                            axis=mybir.AxisListType.X,
                                            op=mybir.AluOpType.max)
                g1T = mlps.tile([128, 12, GB], f32)
                for j in range(12):
                    w = min(128, 1500 - j * 128)
                    ps = mlpp.tile([128, GB], f32, space="PSUM", tag="g1")
                    nc.tensor.matmul(out=ps[:w, :], lhsT=Wg1T_t[:, j * 128:j * 128 + w],
                                     rhs=gT[:], start=True, stop=True)
                    if w < 128:
                        nc.vector.memset(g1T[:, j, :], 0.0)
                    nc.scalar.activation(out=g1T[:w, j, :], in_=ps[:w, :],
                                         func=relu_f, bias=bg1_t[:w, j:j + 1])
                g2ps = mlpp.tile([128, GB], f32, space="PSUM", tag="g2")
                for j in range(12):
                    nc.tensor.matmul(out=g2ps[:], lhsT=Wg2_t[:, j * 128:(j + 1) * 128],
                                     rhs=g1T[:, j, :], start=(j == 0), stop=(j == 11),
                                     skip_group_check=True)
                g2sb = mlps.tile([128, GB], f32)
                nc.vector.tensor_scalar_add(out=g2sb[:], in0=g2ps[:],
                                            scalar1=bg2_t[:])
                ops = mlpp.tile([1, GB], f32, space="PSUM", tag="o")
                nc.tensor.matmul(out=ops[:], lhsT=WoT_t[:], rhs=g2sb[:],
                                 start=True, stop=True)
                osb = mlps.tile([1, GB], f32)
                nc.vector.tensor_scalar_add(out=osb[:], in0=ops[:],
                                            scalar1=float(bo_const))
                nc.sync.dma_start(out=out_d[:, :], in_=osb[:])

    nc.compile()
    return nc


# --------------------------------------------------------------------------
# entry point
# --------------------------------------------------------------------------

_CACHE = {}


def kernel(**inputs):
    from concourse.bass_utils import run_bass_kernel_spmd

    x = np.asarray(inputs['x'], np.float32)
    edge_index = np.asarray(inputs['edge_index'])
    w = _prep_weights(inputs)
    xhT, xl = _prep_x(x)
    gidx, dstrel, recipT, S = _prep_edges(edge_index)

    key = ('prog', S, w['bo'])
    if key not in _CACHE:
        _CACHE[key] = _build_program(S, w['bo'])
    nc = _CACHE[key]

    iota = np.ascontiguousarray(
        np.broadcast_to(np.arange(128, dtype=np.float32), (128, 128)))
    ident = np.eye(128, dtype=np.float32)

    common = dict(
        xhT=xhT, Wq_h=w['Wq_h'], Wkn_h=w['Wkn_h'], Wv_h=w['Wv_h'],
        Wv_aug=w['Wv_aug'], WllT=w['WllT'], WlrT=w['WlrT'], bll=w['bll'],
        Wg1T=w['Wg1T'], bg1=w['bg1'], Wg2Tr=w['Wg2Tr'], bg2=w['bg2'],
        WoT=w['WoT'], iota=iota, ident=ident)
    in_maps = []
    for c in range(NCORE):
        m = dict(common)
        m['xlT'] = xl[c]
        m['gidx'] = gidx[c]
        m['dstrel'] = dstrel[c]
        m['recipT'] = recipT[c]
        in_maps.append(m)

    res = run_bass_kernel_spmd(nc, in_maps, list(range(NCORE)))
    global LAST_RESULT
    LAST_RESULT = res
    out = np.zeros((B, 1), np.float32)
    for c in range(NCORE):
        out[c * GB:(c + 1) * GB, 0] = res.results[c]['out8'].reshape(-1)
    return out


LAST_RESULT = None


# revision 11
# speedup vs baseline: 1.9484x; 1.9484x over previous
"""Trainium2 Bass kernel for nn_GAT_GraphSAGE (N=12000, E=192000, F=35, B=64).

Sharding: the attention "row" dimension (K_new index i, which is also the
softmax row) is sharded 1500 rows/core across 8 cores.  Q and V are computed
replicated (cheap [N,35] projections), so the only collective is one
AllGather of the post-attention node features h.  SAGEConv is sharded by dst
node (same 1500-row shard): h[src] rows are fetched with one batched
dma_gather and scatter-added via one-hot matmuls in PSUM.  Global max-pool +
MLP head run per-core on that core's 8 graphs (graph boundaries align with
the 1500-row shard exactly).

The whole K-branch (Wk/conv-center-taps/Wl/1-sqrt(F)) folds on the host into
a single affine [35->35] map; biases ride an appended ones-row of x^T.
"""
import math
import numpy as np

N, E, F, B = 12000, 192000, 35, 64
NCORE = 8
ROWS = N // NCORE            # 1500
ICH = 512
NI = 3
IPAD = ICH * NI              # 1536
JT = 94                      # j chunks of 128
JPAD = JT * 128              # 12032
XW = 12064                   # padded x^T width (covers 7*1500 + 1536)
DBLK = 12                    # dst blocks (128 each) per core
GB = B // NCORE              # 8 graphs per core
HPAD = 64                    # h row padded to 64 f32 (256B) for dma_gather
GRAPH_BOUNDS = [int(math.ceil(g * (N / B))) for g in range(GB + 1)]
F1 = F + 1


# --------------------------------------------------------------------------
# host-side preprocessing
# --------------------------------------------------------------------------

def _prep_weights(p):
    f64 = np.float64
    f32 = np.float32
    Wq, bq = p['Wq'].astype(f64), p['bq'].astype(f64)
    Wk, bk = p['Wk'].astype(f64), p['bk'].astype(f64)
    Wv, bv = p['Wv'].astype(f64), p['bv'].astype(f64)
    W3c, b3 = p['W3'][:, :, 1].astype(f64), p['b3'].astype(f64)
    W5c, b5 = p['W5'][:, :, 2].astype(f64), p['b5'].astype(f64)
    Wl, bl = p['Wl'].astype(f64), p['bl'].astype(f64)
    Wl1, Wl2, Wl3 = Wl[:, :F], Wl[:, F:2 * F], Wl[:, 2 * F:]

    Weff = W3c.T @ Wl1.T + W5c.T @ Wl2.T + Wl3.T
    beff = b3 @ Wl1.T + b5 @ Wl2.T + bl
    Wkn = Wk.T @ Weff
    bkn = bk @ Weff + beff
    s = 1.0 / np.sqrt(F)
    Wkn, bkn = Wkn * s, bkn * s

    out = {}
    out['Wq_h'] = np.vstack([Wq.T, bq[None, :]]).astype(f32)
    out['Wkn_h'] = np.vstack([Wkn, bkn[None, :]]).astype(f32)
    out['Wv_h'] = np.vstack([Wv.T, bv[None, :]]).astype(f32)
    wva = np.zeros((F1, F1))
    wva[:, :F] = out['Wv_h']
    wva[F, F] = 1.0
    out['Wv_aug'] = wva.astype(f32)
    out['WllT'] = np.ascontiguousarray(p['Wll'].T).astype(f32)
    out['WlrT'] = np.ascontiguousarray(p['Wlr'].T).astype(f32)
    out['bll'] = p['bll'].astype(f32).reshape(F, 1)
    out['Wg1T'] = np.ascontiguousarray(p['Wg1'].T).astype(f32)      # [35,1500]
    bg1 = np.zeros((128, 12), f32)
    bg1.T.reshape(-1)[:1500] = p['bg1'].astype(f32)
    out['bg1'] = bg1
    w2 = np.zeros((12 * 128, 128), f32)
    w2[:1500, :] = p['Wg2'].T.astype(f32)
    out['Wg2Tr'] = np.ascontiguousarray(
        w2.reshape(12, 128, 128).transpose(1, 0, 2).reshape(128, 12 * 128))
    out['bg2'] = p['bg2'].astype(f32).reshape(128, 1)
    out['WoT'] = p['Wo'].astype(f32).reshape(1, 128).T.copy()        # [128,1]
    out['bo'] = float(np.asarray(p['bo']).reshape(-1)[0])
    return out


def _prep_x(x):
    xhT = np.zeros((F1, XW), np.float32)
    xhT[:F, :N] = np.asarray(x, np.float32).T
    xhT[F, :] = 1.0
    xl = [np.ascontiguousarray(xhT[:, c * ROWS: c * ROWS + IPAD])
          for c in range(NCORE)]
    return xhT, xl


def _prep_edges(edge_index):
    src = np.asarray(edge_index[0], np.int64)
    dst = np.asarray(edge_index[1], np.int64)
    deg = np.bincount(dst, minlength=N).astype(np.float64)
    recip = (1.0 / np.maximum(deg, 1.0)).astype(np.float32)

    core_of = dst // ROWS
    blk_of = (dst - core_of * ROWS) // 128
    counts = np.zeros((NCORE, DBLK), np.int64)
    np.add.at(counts, (core_of, blk_of), 1)
    S = int(np.ceil(counts.max() / 128))

    order = np.lexsort((dst,))
    src_s, dst_s = src[order], dst[order]
    core_s, blk_s = core_of[order], blk_of[order]

    gidx, dstrel = [], []
    for c in range(NCORE):
        idx_c = np.zeros(DBLK * S * 128, np.int16)
        rel_c = np.full(DBLK * S * 128, -1.0, np.float32)
        m_c = core_s == c
        for b in range(DBLK):
            m = m_c & (blk_s == b)
            n = int(m.sum())
            lo = b * S * 128
            idx_c[lo:lo + n] = src_s[m].astype(np.int16)
            rel_c[lo:lo + n] = (dst_s[m] - c * ROWS - b * 128).astype(np.float32)
        # HW convention: the [16, n] packed index block must be replicated
        # across all eight 16-partition groups (sim reads only rows 0:16).
        gidx.append(np.ascontiguousarray(
            np.tile(idx_c.reshape(-1, 16).T, (8, 1))))
        dstrel.append(np.ascontiguousarray(rel_c.reshape(-1, 128).T))

    recipT = []
    for c in range(NCORE):
        r = np.ones(IPAD, np.float32)
        r[:ROWS] = recip[c * ROWS:(c + 1) * ROWS]
        recipT.append(np.ascontiguousarray(np.broadcast_to(r, (F, IPAD))))
    return gidx, dstrel, recipT, S


# --------------------------------------------------------------------------
# device program
# --------------------------------------------------------------------------

def _build_program(S, bo_const, timeline=False):
    """timeline=True builds a 1-core variant with the AllGather replaced by
    plain DMA copies — only for TimelineSim cost estimation."""
    import concourse.bass as bass
    import concourse.tile as tile
    from concourse import bacc, mybir

    f32 = mybir.dt.float32
    nc = bacc.Bacc("TRN2", target_bir_lowering=False, debug=False,
                   num_devices=1 if timeline else NCORE)

    din = {}

    def dram_in(name, shape, dt=f32):
        din[name] = nc.dram_tensor(name, list(shape), dt, kind="ExternalInput")
        return din[name]

    xhT_d = dram_in("xhT", (F1, XW))
    xlT_d = dram_in("xlT", (F1, IPAD))
    Wq_d = dram_in("Wq_h", (F1, F))
    Wkn_d = dram_in("Wkn_h", (F1, F))
    Wv_d = dram_in("Wv_h", (F1, F))
    Wva_d = dram_in("Wv_aug", (F1, F1))
    WllT_d = dram_in("WllT", (F, F))
    WlrT_d = dram_in("WlrT", (F, F))
    bll_d = dram_in("bll", (F, 1))
    Wg1T_d = dram_in("Wg1T", (F, 1500))
    bg1_d = dram_in("bg1", (128, 12))
    Wg2_d = dram_in("Wg2Tr", (128, 12 * 128))
    bg2_d = dram_in("bg2", (128, 1))
    WoT_d = dram_in("WoT", (128, 1))
    recT_d = dram_in("recipT", (F, IPAD))
    iota_d = dram_in("iota", (128, 128))
    ident_d = dram_in("ident", (128, 128))
    gidx_d = dram_in("gidx", (128, DBLK * S * 8), mybir.dt.int16)
    drel_d = dram_in("dstrel", (128, DBLK * S))

    out_d = nc.dram_tensor("out8", [1, GB], f32, kind="ExternalOutput")

    h_loc = nc.dram_tensor("h_loc", [ROWS, HPAD], f32)
    h_full = nc.dram_tensor("h_full", [N, HPAD], f32, addr_space="Shared")

    NIDX = DBLK * S * 128

    with tile.TileContext(nc) as tc:
        with tc.tile_pool(name="const", bufs=1) as constp, \
             tc.tile_pool(name="main", bufs=1) as main:
            # ---- constants / small weights ----
            Wq_t = constp.tile([F1, F], f32)
            nc.sync.dma_start(out=Wq_t[:], in_=Wq_d[:, :])
            Wkn_t = constp.tile([F1, F], f32)
            nc.sync.dma_start(out=Wkn_t[:], in_=Wkn_d[:, :])
            Wv_t = constp.tile([F1, F], f32)
            nc.sync.dma_start(out=Wv_t[:], in_=Wv_d[:, :])
            Wva_t = constp.tile([F1, F1], f32)
            nc.sync.dma_start(out=Wva_t[:], in_=Wva_d[:, :])
            ident_t = constp.tile([128, 128], f32)
            nc.sync.dma_start(out=ident_t[:], in_=ident_d[:, :])

            # ---- big persistent sbuf tensors ----
            QT = main.tile([F, JPAD], f32)
            KnT = main.tile([F, IPAD], f32)
            Vp = main.tile([128, JT, F1], f32)          # V' natural, full
            Vl = main.tile([128, DBLK, F], f32)         # V natural, local rows
            hnat = main.tile([128, DBLK, HPAD], f32)    # h natural, local rows

            with tc.tile_pool(name="prep", bufs=2, space="PSUM") as pp, \
                 tc.tile_pool(name="prepin", bufs=1) as pin, \
                 tc.tile_pool(name="prepsb", bufs=3) as psb:
                xhT_t = pin.tile([F1, XW], f32)
                nc.sync.dma_start(out=xhT_t[:], in_=xhT_d[:, :])
                xlT_t = pin.tile([F1, IPAD], f32)
                nc.sync.dma_start(out=xlT_t[:], in_=xlT_d[:, :])

                # QT full: 24 matmuls over 512-chunks (covers JPAD=12032)
                nq = JPAD // ICH  # 23.5 -> handle tail
                for ci in range((JPAD + ICH - 1) // ICH):
                    w = min(ICH, JPAD - ci * ICH)
                    ps = pp.tile([F, ICH], f32, space="PSUM", tag="ppq")
                    nc.tensor.matmul(out=ps[:, :w], lhsT=Wq_t[:],
                                     rhs=xhT_t[:, ci * ICH: ci * ICH + w],
                                     start=True, stop=True)
                    nc.vector.tensor_copy(out=QT[:, ci * ICH: ci * ICH + w],
                                          in_=ps[:, :w])
                # K_newT local
                for ci in range(NI):
                    ps = pp.tile([F, ICH], f32, space="PSUM", tag="ppq")
                    nc.tensor.matmul(out=ps[:], lhsT=Wkn_t[:],
                                     rhs=xlT_t[:, ci * ICH:(ci + 1) * ICH],
                                     start=True, stop=True)
                    nc.vector.tensor_copy(out=KnT[:, ci * ICH:(ci + 1) * ICH],
                                          in_=ps[:])
                # V natural local rows (12 x [128,35])
                for t in range(DBLK):
                    ps = pp.tile([128, F], f32, space="PSUM", tag="ppv")
                    nc.tensor.matmul(out=ps[:], lhsT=xlT_t[:, t * 128:(t + 1) * 128],
                                     rhs=Wv_t[:], start=True, stop=True)
                    nc.vector.tensor_copy(out=Vl[:, t, :], in_=ps[:])
                # V' natural full (94 x [128,36]); zero the 32 pad rows of
                # the last chunk (j in [12000,12032))
                for j in range(JT):
                    ps = pp.tile([128, F1], f32, space="PSUM", tag="ppv")
                    nc.tensor.matmul(out=ps[:], lhsT=xhT_t[:, j * 128:(j + 1) * 128],
                                     rhs=Wva_t[:], start=True, stop=True)
                    if j == JT - 1:
                        nc.vector.tensor_copy(out=Vp[:96, j, :], in_=ps[:96, :])
                        nc.vector.memset(Vp[96:128, j, :], 0.0)
                    else:
                        nc.vector.tensor_copy(out=Vp[:, j, :], in_=ps[:])

            # ---------------- attention ----------------
            # j-chunks processed in groups of 3 so one ACT exp instruction
            # covers [128, 1536] (3 PSUM banks) — amortizes the ~352-cycle
            # per-ACTIVATE overhead, ACT being the bottleneck engine.
            GROUPS = [(g * 3, 3) for g in range(JT // 3)]
            if JT % 3:
                GROUPS.append((JT - JT % 3, JT % 3))
            with tc.tile_pool(name="mm1p", bufs=2, space="PSUM") as mm1p, \
                 tc.tile_pool(name="Up", bufs=1, space="PSUM") as Upp, \
                 tc.tile_pool(name="tp", bufs=1, space="PSUM") as tpp, \
                 tc.tile_pool(name="esb", bufs=3) as esb, \
                 tc.tile_pool(name="usb", bufs=2) as usb, \
                 tc.tile_pool(name="hsmall", bufs=4) as hsmall:
                exp_f = mybir.ActivationFunctionType.Exp
                for ci in range(NI):
                    Ups = Upp.tile([F1, ICH], f32, space="PSUM", tag="U")
                    prev = None  # (exp_tile, j0, glen)
                    for (j0, glen) in GROUPS:
                        ps = mm1p.tile([128, 3 * ICH], f32, space="PSUM", tag="s")
                        for k in range(glen):
                            j = j0 + k
                            nc.tensor.matmul(
                                out=ps[:, k * ICH:(k + 1) * ICH],
                                lhsT=QT[:, j * 128:(j + 1) * 128],
                                rhs=KnT[:, ci * ICH:(ci + 1) * ICH],
                                start=True, stop=True)
                        et = esb.tile([128, 3 * ICH], f32, tag="e")
                        nc.scalar.activation(out=et[:, :glen * ICH],
                                             in_=ps[:, :glen * ICH], func=exp_f)
                        if prev is not None:
                            pe, pj0, pglen = prev
                            for k in range(pglen):
                                nc.tensor.matmul(
                                    out=Ups[:], lhsT=Vp[:, pj0 + k, :],
                                    rhs=pe[:, k * ICH:(k + 1) * ICH],
                                    start=(pj0 + k == 0), stop=False,
                                    skip_group_check=True)
                        prev = (et, j0, glen)
                    pe, pj0, pglen = prev
                    for k in range(pglen):
                        nc.tensor.matmul(out=Ups[:], lhsT=Vp[:, pj0 + k, :],
                                         rhs=pe[:, k * ICH:(k + 1) * ICH],
                                         start=False,
                                         stop=(k == pglen - 1),
                                         skip_group_check=True)
                    # normalize + residual + relu -> h natural tiles
                    Usb = usb.tile([F1, ICH], f32, tag="usb")
                    nc.vector.tensor_copy(out=Usb[:], in_=Ups[:])
                    for t in range(4):
                        blk = ci * 4 + t
                        up = tpp.tile([128, F1], f32, space="PSUM", tag="unat")
                        nc.tensor.transpose(out=up[:], in_=Usb[:, t * 128:(t + 1) * 128],
                                            identity=ident_t[:F1, :F1])
                        rec = hsmall.tile([128, 1], f32, tag="rec")
                        nc.vector.reciprocal(out=rec[:], in_=up[:, F:F1])
                        hh = hsmall.tile([128, F], f32, tag="hh")
                        nc.vector.scalar_tensor_tensor(
                            out=hh[:], in0=up[:, :F], scalar=rec[:],
                            in1=Vl[:, blk, :], op0=mybir.AluOpType.mult,
                            op1=mybir.AluOpType.add)
                        nc.vector.tensor_scalar_max(out=hnat[:, blk, :F], in0=hh[:],
                                                    scalar1=0.0)
                        nc.vector.memset(hnat[:, blk, F:HPAD], 0.0)
                        # store valid rows to DRAM for the AllGather
                        lo = blk * 128
                        nrows = min(128, max(0, ROWS - lo))
                        if nrows > 0:
                            nc.sync.dma_start(
                                out=h_loc[lo:lo + nrows, :],
                                in_=hnat[:nrows, blk, :])

            # hT local (for SAGE lin_r): transpose the 12 h tiles
            hT = main.tile([F, IPAD], f32)
            with tc.tile_pool(name="htp", bufs=2, space="PSUM") as htp:
                for t in range(DBLK):
                    ps = htp.tile([F, 128], f32, space="PSUM", tag="ht")
                    nc.tensor.transpose(out=ps[:], in_=hnat[:, t, :F],
                                        identity=ident_t[:])
                    nc.vector.tensor_copy(out=hT[:, t * 128:(t + 1) * 128], in_=ps[:])

            # ---------------- AllGather h ----------------
            if timeline:
                for c in range(NCORE):
                    nc.sync.dma_start(out=h_full[c * ROWS:(c + 1) * ROWS, :],
                                      in_=h_loc[:, :])
            else:
                nc.gpsimd.collective_compute(
                    "AllGather", mybir.AluOpType.bypass,
                    replica_groups=[list(range(NCORE))],
                    ins=[h_loc[:, :]], outs=[h_full[:, :]])

            # ---------------- SAGE scatter ----------------
            aggdT = main.tile([F, IPAD], f32)
            h2T = main.tile([F, IPAD], f32)
            with tc.tile_pool(name="gat", bufs=1) as gat, \
                 tc.tile_pool(name="sca", bufs=4) as sca, \
                 tc.tile_pool(name="scp", bufs=2, space="PSUM") as scp, \
                 tc.tile_pool(name="sin", bufs=1) as sin:
                iota_t = sin.tile([128, 128], f32)
                nc.sync.dma_start(out=iota_t[:], in_=iota_d[:, :])
                drel_t = sin.tile([128, DBLK * S], f32)
                nc.sync.dma_start(out=drel_t[:], in_=drel_d[:, :])
                idx_t = sin.tile([128, DBLK * S * 8], mybir.dt.int16)
                nc.sync.dma_start(out=idx_t[:], in_=gidx_d[:, :])
                recT_t = sin.tile([F, IPAD], f32)
                nc.sync.dma_start(out=recT_t[:], in_=recT_d[:, :])

                G = gat.tile([128, DBLK * S, HPAD], f32)
                # split the gather to pipeline SWDGE/SDMA with the matmuls
                GSPLIT = 4
                assert (DBLK * S) % GSPLIT == 0
                cpg = DBLK * S // GSPLIT          # chunks per gather
                for g in range(GSPLIT):
                    nc.gpsimd.dma_gather(
                        out_ap=G[:, g * cpg:(g + 1) * cpg, :],
                        in_ap=h_full[:, :],
                        idxs_ap=idx_t[:, g * cpg * 8:(g + 1) * cpg * 8],
                        num_idxs=cpg * 128,
                        num_idxs_reg=cpg * 128,
                        elem_size=HPAD,
                        single_packet=False)

                for b in range(DBLK):
                    acc = scp.tile([F, 128], f32, space="PSUM", tag="agg")
                    for s in range(S):
                        ch = b * S + s
                        P = sca.tile([128, 128], f32, tag="P")
                        nc.vector.tensor_scalar(
                            out=P[:], in0=iota_t[:],
                            scalar1=drel_t[:, ch:ch + 1], scalar2=None,
                            op0=mybir.AluOpType.is_equal)
                        nc.tensor.matmul(out=acc[:], lhsT=G[:, ch, :F], rhs=P[:],
                                         start=(s == 0), stop=(s == S - 1),
                                         skip_group_check=True)
                    nc.vector.tensor_mul(out=aggdT[:, b * 128:(b + 1) * 128],
                                         in0=acc[:],
                                         in1=recT_t[:, b * 128:(b + 1) * 128])

            # ---------------- SAGE linear + pool + MLP ----------------
            with tc.tile_pool(name="mlpw", bufs=1) as mlpw, \
                 tc.tile_pool(name="mlps", bufs=2) as mlps, \
                 tc.tile_pool(name="mlpp", bufs=2, space="PSUM") as mlpp:
                WllT_t = mlpw.tile([F, F], f32)
                nc.sync.dma_start(out=WllT_t[:], in_=WllT_d[:, :])
                WlrT_t = mlpw.tile([F, F], f32)
                nc.sync.dma_start(out=WlrT_t[:], in_=WlrT_d[:, :])
                bll_t = mlpw.tile([F, 1], f32)
                nc.sync.dma_start(out=bll_t[:], in_=bll_d[:, :])
                Wg1T_t = mlpw.tile([F, 1500], f32)
                nc.sync.dma_start(out=Wg1T_t[:], in_=Wg1T_d[:, :])
                bg1_t = mlpw.tile([128, 12], f32)
                nc.sync.dma_start(out=bg1_t[:], in_=bg1_d[:, :])
                Wg2_t = mlpw.tile([128, 12 * 128], f32)
                nc.sync.dma_start(out=Wg2_t[:], in_=Wg2_d[:, :])
                bg2_t = mlpw.tile([128, 1], f32)
                nc.sync.dma_start(out=bg2_t[:], in_=bg2_d[:, :])
                WoT_t = mlpw.tile([128, 1], f32)
                nc.sync.dma_start(out=WoT_t[:], in_=WoT_d[:, :])

                relu_f = mybir.ActivationFunctionType.Relu
                for ci in range(NI):
                    ps = mlpp.tile([F, ICH], f32, space="PSUM", tag="h2")
                    nc.tensor.matmul(out=ps[:], lhsT=WllT_t[:],
                                     rhs=aggdT[:, ci * ICH:(ci + 1) * ICH],
                                     start=True, stop=False, skip_group_check=True)
                    nc.tensor.matmul(out=ps[:], lhsT=WlrT_t[:],
                                     rhs=hT[:, ci * ICH:(ci + 1) * ICH],
                                     start=False, stop=True, skip_group_check=True)
                    nc.scalar.activation(out=h2T[:, ci * ICH:(ci + 1) * ICH],
                                         in_=ps[:], func=relu_f, bias=bll_t[:])

                gT = mlps.tile([F, GB], f32)
                for g in range(GB):
                    lo, hi = GRAPH_BOUNDS[g], GRAPH_BOUNDS[g + 1]
                    nc.vector.tensor_reduce(out=gT[:, g:g + 1], in_=h2T[:, lo:hi],
                                            axis=mybir.AxisListType.X,
                                            op=mybir.AluOpType.max)
                g1T = mlps.tile([128, 12, GB], f32)
                for j in range(12):
                    w = min(128, 1500 - j * 128)
                    ps = mlpp.tile([128, GB], f32, space="PSUM", tag="g1")
                    nc.tensor.matmul(out=ps[:w, :], lhsT=Wg1T_t[:, j * 128:j * 128 + w],
                                     rhs=gT[:], start=True, stop=True)
                    if w < 128:
                        nc.vector.memset(g1T[:, j, :], 0.0)
                    nc.scalar.activation(out=g1T[:w, j, :], in_=ps[:w, :],
                                         func=relu_f, bias=bg1_t[:w, j:j + 1])
                g2ps = mlpp.tile([128, GB], f32, space="PSUM", tag="g2")
                for j in range(12):
                    nc.tensor.matmul(out=g2ps[:], lhsT=Wg2_t[:, j * 128:(j + 1) * 128],
                                     rhs=g1T[:, j, :], start=(j == 0), stop=(j == 11),
                                     skip_group_check=True)
                g2sb = mlps.tile([128, GB], f32)
                nc.vector.tensor_scalar_add(out=g2sb[:], in0=g2ps[:],
                                            scalar1=bg2_t[:])
                ops = mlpp.tile([1, GB], f32, space="PSUM", tag="o")
                nc.tensor.matmul(out=ops[:], lhsT=WoT_t[:], rhs=g2sb[:],
                                 start=True, stop=True)
                osb = mlps.tile([1, GB], f32)
                nc.vector.tensor_scalar_add(out=osb[:], in0=ops[:],
                                            scalar1=float(bo_const))
                nc.sync.dma_start(out=out_d[:, :], in_=osb[:])

    nc.compile()
    return nc


# --------------------------------------------------------------------------
# entry point
# --------------------------------------------------------------------------

_CACHE = {}


def kernel(**inputs):
    from concourse.bass_utils import run_bass_kernel_spmd

    x = np.asarray(inputs['x'], np.float32)
    edge_index = np.asarray(inputs['edge_index'])
    w = _prep_weights(inputs)
    xhT, xl = _prep_x(x)
    gidx, dstrel, recipT, S = _prep_edges(edge_index)

    key = ('prog', S, w['bo'])
    if key not in _CACHE:
        _CACHE[key] = _build_program(S, w['bo'])
    nc = _CACHE[key]

    iota = np.ascontiguousarray(
        np.broadcast_to(np.arange(128, dtype=np.float32), (128, 128)))
    ident = np.eye(128, dtype=np.float32)

    common = dict(
        xhT=xhT, Wq_h=w['Wq_h'], Wkn_h=w['Wkn_h'], Wv_h=w['Wv_h'],
        Wv_aug=w['Wv_aug'], WllT=w['WllT'], WlrT=w['WlrT'], bll=w['bll'],
        Wg1T=w['Wg1T'], bg1=w['bg1'], Wg2Tr=w['Wg2Tr'], bg2=w['bg2'],
        WoT=w['WoT'], iota=iota, ident=ident)
    in_maps = []
    for c in range(NCORE):
        m = dict(common)
        m['xlT'] = xl[c]
        m['gidx'] = gidx[c]
        m['dstrel'] = dstrel[c]
        m['recipT'] = recipT[c]
        in_maps.append(m)

    res = run_bass_kernel_spmd(nc, in_maps, list(range(NCORE)))
    global LAST_RESULT
    LAST_RESULT = res
    out = np.zeros((B, 1), np.float32)
    for c in range(NCORE):
        out[c * GB:(c + 1) * GB, 0] = res.results[c]['out8'].reshape(-1)
    return out


LAST_RESULT = None
